# revision 19
# baseline (speedup 1.0000x reference)
"""DisKT forward kernel for 8 Trainium2 NeuronCores.

Strategy: pure data-parallel over batch (32 samples -> 4 per core).
All activations bf16, matmul accumulation fp32 in PSUM.

Layouts:
  FEATURE tile [128, 4, 512]: [:, c, s] = feature dims c*128..(c+1)*128, token s.
  TOKEN   tile [128, 4, 512]: [:, tc, d] = tokens tc*128.., feature d.
Matmuls contract over the partition dim:
  feature->feature linear: lhsT = W[kchunk, mblock], rhs = x[:, kchunk, :]
  feature->token   linear: lhsT = x[:, kchunk, sblock], rhs = W[kchunk]
Attention runs in scoresT [j, i] layout with a max-free softmax (scores are
tiny), masked entries exactly zero, denominator via ones-column matmuls.
"""
import numpy as np
import ml_dtypes

import concourse.bass as bass
import concourse.bacc as bacc
import concourse.mybir as mybir
from concourse import tile
from concourse.bass_utils import run_bass_kernel_spmd

F32 = mybir.dt.float32
BF16 = mybir.dt.bfloat16
I32 = mybir.dt.int32
OP = mybir.AluOpType
AF = mybir.ActivationFunctionType

B, S, D, H, NB = 32, 512, 512, 8, 4
DH = D // H          # 64
NC_CORES = 8
BPC = B // NC_CORES  # 4 samples per core
C = D // 128         # 4 feature chunks
TC = S // 128        # 4 token chunks

bf16 = ml_dtypes.bfloat16


# ---------------------------------------------------------------- host prep --
class ConstPack:
    """[128, N] fp32 tile of per-partition constants, addressed by name+chunk."""

    def __init__(self):
        self.cols = []
        self.index = {}

    def add(self, name, vec):
        vec = np.asarray(vec, np.float32).reshape(-1)
        assert vec.size % 128 == 0
        self.index[name] = len(self.cols)
        for c in range(vec.size // 128):
            self.cols.append(vec[c * 128:(c + 1) * 128])

    def array(self):
        return np.stack(self.cols, axis=1).astype(np.float32)


def _posenc():
    pos = np.arange(S, dtype=np.float32)[:, None]
    div = np.exp(np.arange(0, D, 2, dtype=np.float32) * (-np.log(10000.0) / D))
    pe = np.zeros((S, D), np.float32)
    pe[:, 0::2] = np.sin(pos * div)
    pe[:, 1::2] = np.cos(pos * div)
    return pe


def prep_host(params):
    g = {}
    tonp = lambda a: np.asarray(a, np.float32)
    blocks = params["blocks"]
    for l, bp in enumerate(blocks):
        for w in ("Wq", "Wk", "Wv", "Wo", "W1", "W2"):
            g[f"{w.lower()}{l}"] = tonp(bp[w]).astype(bf16)
    g["ffw1"] = tonp(params["ffn"]["w1"]).astype(bf16)
    g["ffw2"] = tonp(params["ffn"]["w2"]).astype(bf16)
    g["pw1"] = tonp(params["pred"]["w1"]).astype(bf16)
    g["pw2"] = tonp(params["pred"]["w2"]).astype(bf16)
    g["tb_c"] = tonp(params["concept"]).astype(bf16)
    g["tb_cv"] = tonp(params["concept_var"]).astype(bf16)
    g["tb_qd"] = tonp(params["question_diff"]).astype(bf16)

    cp = ConstPack()
    for l, bp in enumerate(blocks):
        cp.add(f"bq{l}", tonp(bp["bq"]))
        cp.add(f"bk{l}", tonp(bp["bk"]))
        cp.add(f"bo{l}", tonp(bp["bo"]))
        cp.add(f"b1_{l}", tonp(bp["b1"]))
        cp.add(f"b2_{l}", tonp(bp["b2"]))
        for ln in ("ln1", "ln2"):
            gv, bv_ = tonp(bp[f"{ln}_g"]), tonp(bp[f"{ln}_b"])
            cp.add(f"{ln}g{l}", gv)
            cp.add(f"{ln}gn{l}", -gv / 512.0)
            cp.add(f"{ln}b{l}", bv_)
    cp.add("ffb1", tonp(params["ffn"]["b1"]))
    cp.add("pb1", tonp(params["pred"]["b1"]))
    inter = tonp(params["interaction"])
    ivar = tonp(params["interaction_var"])
    cp.add("eps5", np.full(128, 1e-5))
    cp.add("eps12", np.full(128, 1e-12))
    cp.add("ivar0", ivar[0])
    cp.add("ivar1", ivar[1])
    cp.add("inter0", inter[0])
    cp.add("inter1", inter[1])
    g["cpack"] = cp.array()
    g["_cpidx"] = cp.index

    c0 = tonp(params["concept"])[0]
    qd0 = tonp(params["question_diff"])[0]
    const_p = inter[2] + c0 + qd0 * ivar[2]

    g["cprow"] = const_p.reshape(1, 512).astype(bf16)

    rowbank = np.zeros((1, 8 * 512), np.float32)
    for l, bp in enumerate(blocks):
        rowbank[0, l * 512:(l + 1) * 512] = tonp(bp["bv"])
    rowbank[0, 4 * 512:5 * 512] = tonp(params["ffn"]["b2"])
    rowbank[0, 5 * 512:6 * 512] = tonp(params["ffn"]["ln_g"])
    rowbank[0, 6 * 512:7 * 512] = tonp(params["ffn"]["ln_b"])
    g["rowbank"] = rowbank.astype(bf16)

    # ecols[:, h*8+j] = (j == h): indicator columns for M=8 row-select matmuls
    ecols = np.zeros((128, 64), np.float32)
    for h in range(8):
        ecols[:, h * 8 + h] = 1.0
    g["ecols"] = ecols.astype(bf16)

    # sel8[hp*8+k, m] = 1 if k == 2*hp + (m//64): rcp row broadcast per pair
    sel8 = np.zeros((32, 128), np.float32)
    for hp in range(4):
        for m in range(128):
            sel8[hp * 8 + 2 * hp + (m // 64), m] = 1.0
    g["sel8"] = sel8.astype(bf16)

    g["peT"] = _posenc().T.copy().astype(bf16)
    g["ident"] = np.eye(128, dtype=np.float32).astype(bf16)
    g["allones"] = np.ones((128, 128), np.float32).astype(bf16)
    g["trimask"] = (np.arange(128)[:, None] < np.arange(128)[None, :]).astype(np.float32).astype(bf16)
    g["pb2"] = tonp(params["pred"]["b2"]).reshape(1, 1)
    return g


def prep_core(question_seq, concept_seq, correctness_seq, counter_mask_seq, core):
    sl = slice(core * BPC, (core + 1) * BPC)
    q = np.asarray(question_seq)[sl].astype(np.int32)
    c = np.asarray(concept_seq)[sl].astype(np.int32)
    r = np.asarray(correctness_seq)[sl].astype(np.int32)
    cm = np.asarray(counter_mask_seq)[sl].astype(np.int32)

    d = {}
    d["rbank"] = r.astype(np.float32).reshape(1, -1).astype(bf16)
    d["ombank"] = (1 - r).astype(np.float32).reshape(1, -1).astype(bf16)
    cols = lambda a, dt: a.reshape(BPC, TC, 128).transpose(2, 0, 1).reshape(128, BPC * TC).astype(dt)
    d["ccols"] = cols(cm.astype(np.float32), np.float32)
    d["icol_c"] = cols(c, np.int32)
    d["icol_q"] = cols(q, np.int32)
    return d


# ------------------------------------------------------------------- build --
def build(cpidx, ncols):
    nc = bacc.Bacc("TRN2", target_bir_lowering=False, debug=False,
                   num_devices=NC_CORES)
    P = {}

    def param(name, shape, dtype):
        P[name] = nc.declare_dram_parameter(name, list(shape), dtype, isOutput=False)

    for l in range(NB):
        for w, sh in (("wq", (D, D)), ("wk", (D, D)), ("wv", (D, D)),
                      ("wo", (D, D)), ("w1", (D, 2 * D)), ("w2", (2 * D, D))):
            param(f"{w}{l}", sh, BF16)
    param("ffw1", (D, 2 * D), BF16)
    param("ffw2", (2 * D, D), BF16)
    param("pw1", (3 * D, D), BF16)
    param("pw2", (D, 1), BF16)
    param("tb_c", (1000, D), BF16)
    param("tb_cv", (1000, D), BF16)
    param("tb_qd", (10000, D), BF16)
    param("cpack", (128, ncols), F32)
    param("cprow", (1, 512), BF16)
    param("rowbank", (1, 8 * 512), BF16)
    param("ecols", (128, 64), BF16)
    param("sel8", (32, 128), BF16)
    param("peT", (D, S), BF16)
    param("ident", (128, 128), BF16)
    param("allones", (128, 128), BF16)
    param("trimask", (128, 128), BF16)
    param("pb2", (1, 1), F32)
    param("rbank", (1, BPC * S), BF16)
    param("ombank", (1, BPC * S), BF16)
    param("ccols", (128, BPC * TC), F32)
    param("icol_c", (128, BPC * TC), I32)
    param("icol_q", (128, BPC * TC), I32)

    out = nc.declare_dram_parameter("out", [BPC, S], F32, isOutput=True)

    spill = {}
    for b in range(BPC):
        for nm in ("y1t", "y2t", "qemb", "qdiff"):
            spill[(nm, b)] = nc.dram_tensor(f"sp_{nm}_{b}", [128, C, S], BF16)

    with tile.TileContext(nc) as tc:
        build_body(nc, tc, P, out, spill, cpidx)
    return nc


def build_body(nc, tc, P, out, spill, cpidx):
    from contextlib import ExitStack
    ctx = ExitStack()
    consts = ctx.enter_context(tc.tile_pool(name="consts", bufs=1))
    persist = ctx.enter_context(tc.tile_pool(name="persist", bufs=1))
    wpool = ctx.enter_context(tc.tile_pool(name="wpool", bufs=1))
    work = ctx.enter_context(tc.tile_pool(name="work", bufs=1))
    psp = ctx.enter_context(tc.tile_pool(name="psp", bufs=1, space="PSUM"))

    w_feat = lambda p: p[:].rearrange("(k p) m -> p k m", p=128)

    def dma_const(name, shape, dtype, src_ap=None):
        t = consts.tile(list(shape), dtype, name=f"t_{name}")
        nc.sync.dma_start(out=t[:], in_=src_ap if src_ap is not None else P[name][:])
        return t

    t_cpack = dma_const("cpack", (128, P["cpack"].shape[1]), F32)
    t_cprow = dma_const("cprow", (1, 512), BF16)
    t_rowbank = dma_const("rowbank", (1, 8 * 512), BF16)
    t_ecols = dma_const("ecols", (128, 64), BF16)
    t_sel8 = [dma_const(f"sel8_{hp}", (8, 128), BF16, P["sel8"][hp * 8:(hp + 1) * 8, :])
              for hp in range(4)]
    t_peT = dma_const("peT", (128, C, S), BF16, w_feat(P["peT"]))
    t_ident = dma_const("ident", (128, 128), BF16)
    t_ones = dma_const("allones", (128, 128), BF16)
    t_tri = dma_const("trimask", (128, 128), BF16)
    t_pb2 = dma_const("pb2", (1, 1), F32)
    t_rbank = dma_const("rbank", (1, BPC * S), BF16)
    t_ombank = dma_const("ombank", (1, BPC * S), BF16)
    t_ccols = dma_const("ccols", (128, BPC * TC), F32)
    t_icc = dma_const("icol_c", (128, BPC * TC), I32)
    t_icq = dma_const("icol_q", (128, BPC * TC), I32)

    cp = lambda name, c=0: t_cpack[:, cpidx[name] + c: cpidx[name] + c + 1]
    onesrow = t_ones[0:1, :]
    onescol = t_ones[:, 0:1]
    rowb = lambda i: t_rowbank[0:1, i * 512:(i + 1) * 512]
    rrow = lambda b: t_rbank[0:1, b * 512:(b + 1) * 512]
    omrow = lambda b: t_ombank[0:1, b * 512:(b + 1) * 512]
    ecol = lambda h: t_ecols[:, h * 8:h * 8 + 8]

    # ffn-LN g/b broadcast tiles
    def bcast_row(row_ap, name, pool=None, tag=""):
        ps = psp.tile([128, 512], F32, tag="B", bufs=2, name=f"psbc_{name}")
        nc.tensor.matmul(ps[:], onesrow, row_ap, start=True, stop=True)
        t = (pool or consts).tile([128, 512], BF16, name=name, tag=tag)
        nc.vector.tensor_copy(out=t[:], in_=ps[:])
        return t

    t_ffg_bc = bcast_row(rowb(5), "ffg_bc")
    t_ffb_bc = bcast_row(rowb(6), "ffb_bc")

    X = [persist.tile([128, C, S], BF16, name=f"x_{b}") for b in range(BPC)]
    Y = [persist.tile([128, C, S], BF16, name=f"y_{b}") for b in range(BPC)]

    # ================= helpers =================
    def linear_feat(dst, src, wtile, nk, nm, bias_name=None, relu=False,
                    resid=None, bias_row=None, square_dst=None):
        for m in range(nm):
            ps = psp.tile([128, 512], F32, tag="B", bufs=2, name="pslin")
            nmm = nk + (resid is not None) + (bias_row is not None)
            i = 0
            for k in range(nk):
                i += 1
                nc.tensor.matmul(ps[:], wtile[:, k, m * 128:(m + 1) * 128],
                                 src[:, k, :], start=(i == 1), stop=(i == nmm))
            if resid is not None:
                i += 1
                nc.tensor.matmul(ps[:], t_ident[:], resid[:, m, :],
                                 start=False, stop=(i == nmm))
            if bias_row is not None:
                i += 1
                nc.tensor.matmul(ps[:], onesrow, bias_row, start=False, stop=True)
            use_act = (m % 2 == 1) and square_dst is None
            if relu:
                if use_act:
                    nc.scalar.activation(dst[:, m, :], ps[:], AF.Relu,
                                         bias=cp(bias_name, m))
                else:
                    nc.vector.tensor_scalar(out=dst[:, m, :], in0=ps[:],
                                            scalar1=cp(bias_name, m), scalar2=0.0,
                                            op0=OP.add, op1=OP.max)
            elif bias_name is not None:
                if use_act:
                    nc.scalar.activation(dst[:, m, :], ps[:], AF.Identity,
                                         bias=cp(bias_name, m))
                else:
                    nc.vector.tensor_scalar(out=dst[:, m, :], in0=ps[:],
                                            scalar1=cp(bias_name, m), scalar2=None,
                                            op0=OP.add)
            else:
                if use_act:
                    nc.scalar.copy(dst[:, m, :], ps[:])
                else:
                    nc.vector.tensor_copy(out=dst[:, m, :], in_=ps[:])
            if square_dst is not None:
                nc.scalar.activation(square_dst[:, m, :], ps[:], AF.Square,
                                     bias=cp(bias_name, m))

    def ln_feature(dst, z, zsq, gname, gnname, bname, eps_name):
        s1 = psp.tile([128, 512], F32, tag="C", bufs=2, name="s1")
        s2 = psp.tile([128, 512], F32, tag="C", bufs=2, name="s2")
        for c in range(C):
            nc.tensor.matmul(s1[:], t_ones[:], z[:, c, :], start=(c == 0), stop=(c == C - 1))
        for c in range(C):
            nc.tensor.matmul(s2[:], t_ones[:], zsq[:, c, :], start=(c == 0), stop=(c == C - 1))
        sq = work.tile([128, 512], F32, tag="ln_sq", name="ln_sq")
        nc.scalar.square(sq[:], s1[:])
        s1s = work.tile([128, 512], F32, tag="ln_s1s", name="ln_s1s")
        nc.vector.tensor_copy(out=s1s[:], in_=s1[:])
        v512 = work.tile([128, 512], F32, tag="ln_v", name="ln_v")
        nc.vector.scalar_tensor_tensor(out=v512[:], in0=sq[:], scalar=-1.0 / 512,
                                       in1=s2[:], op0=OP.mult, op1=OP.add)
        sqv = work.tile([128, 512], F32, tag="ln_sq", name="ln_sqv")
        nc.scalar.activation(sqv[:], v512[:], AF.Sqrt, bias=cp(eps_name), scale=1.0 / 512)
        rstd = work.tile([128, 512], F32, tag="ln_v", name="ln_rstd")
        nc.vector.reciprocal_approx_fast(out=rstd[:], in_=sqv[:])
        rstd_bf = work.tile([128, 512], BF16, tag="ln_rstdb", name="ln_rstdb")
        nc.vector.tensor_copy(out=rstd_bf[:], in_=rstd[:])
        for c in range(C):
            bc = work.tile([128, 512], BF16, tag="ln_bc", name="ln_bc")
            nc.vector.scalar_tensor_tensor(out=bc[:], in0=s1s[:], scalar=cp(gnname, c),
                                           in1=rstd[:], op0=OP.mult, op1=OP.mult)
            t = work.tile([128, 512], BF16, tag="ln_t", name="ln_t")
            nc.vector.scalar_tensor_tensor(out=t[:], in0=z[:, c, :], scalar=cp(gname, c),
                                           in1=rstd_bf[:], op0=OP.mult, op1=OP.mult)
            tb2 = work.tile([128, 512], BF16, tag="ln_tb2", name="ln_tb2")
            nc.gpsimd.tensor_tensor(out=tb2[:], in0=t[:], in1=bc[:], op=OP.add)
            nc.gpsimd.tensor_scalar(out=dst[:, c, :], in0=tb2[:], scalar1=cp(bname, c),
                                    scalar2=None, op0=OP.add)

    def attention(xq, xk, vtok, outdst, b, dual_extra=None):
        ntiles = {}
        for hp in range(4):
            for c in range(TC):
                ntiles[(hp, c)] = work.tile([128, 2, 512 - c * 128], BF16,
                                            tag=f"n_{hp}_{c}", name=f"n_{hp}_{c}")
        for hp in range(4):
            for c in range(TC):
                nt = ntiles[(hp, c)]
                ps = psp.tile([128, 2, 512], F32, tag="A", bufs=2, name="ps_sc")
                for hh in range(2):
                    lo = hh * 64
                    nc.tensor.matmul(ps[:, hh, c * 128:],
                                     xk[lo:lo + 64, hp, c * 128:(c + 1) * 128],
                                     xq[lo:lo + 64, hp, c * 128:],
                                     start=True, stop=True)
                nc.scalar.activation(nt[:, :, :], ps[:, :, c * 128:],
                                     AF.Exp, scale=1.0 / np.sqrt(DH))
                mb = t_tri[:].unsqueeze(1).broadcast_to([128, 2, 128])
                nc.vector.tensor_tensor(out=nt[:, :, 0:128],
                                        in0=nt[:, :, 0:128],
                                        in1=mb, op=OP.mult)
                if dual_extra is not None:
                    nc.vector.tensor_scalar(
                        out=nt[:, :, :], in0=nt[:, :, :],
                        scalar1=t_ccols[:, b * TC + c: b * TC + c + 1], scalar2=None,
                        op0=OP.mult)
        den = psp.tile([8, 512], F32, tag="A", bufs=2, name="ps_den")
        nmm_den = H * TC
        i_den = 0
        for h in range(H):
            hp, hh = h // 2, h % 2
            for c in range(TC):
                i_den += 1
                nc.tensor.matmul(den[:, c * 128:], ecol(h),
                                 ntiles[(hp, c)][:, hh, :],
                                 start=(i_den == 1), stop=(i_den == nmm_den))
        det_bf = None
        if dual_extra is None:
            nc.vector.memset(den[0:8, 0:1], 1.0)
            denf = den
        else:
            denf = work.tile([8, 512], F32, tag="at_denf", name="at_denf")
            det8 = work.tile([8, 512], F32, tag="at_det", name="at_det")
            nc.vector.tensor_scalar(out=det8[:], in0=den[:], scalar1=0.0, scalar2=None,
                                    op0=OP.is_equal)
            nc.vector.memset(det8[:, 0:1], 0.0)
            det_bf = work.tile([1, 512], BF16, tag="at_detb", name="at_detb")
            nc.vector.tensor_copy(out=det_bf[:], in_=det8[0:1, :])
            nc.vector.scalar_tensor_tensor(out=denf[:], in0=det8[:], scalar=512.0,
                                           in1=den[:], op0=OP.mult, op1=OP.add)
            nc.vector.memset(denf[:, 0:1], 1.0)
        rcp = work.tile([8, 512], F32, tag="at_rcp", name="at_rcp")
        nc.vector.reciprocal_approx_fast(out=rcp[:], in_=denf[:])
        rcpb = work.tile([8, 512], BF16, tag="at_rcpb", name="at_rcpb")
        nc.vector.tensor_copy(out=rcpb[:], in_=rcp[:])

        streams = [(0, vtok, outdst)]
        if dual_extra is not None:
            streams.append((1, dual_extra["vtok2"], dual_extra["outdst2"]))
        for si, vt, odst in streams:
            def attnv(hp):
                pa = psp.tile([128, 512], F32, tag="B", bufs=2, name="ps_attn")
                for hh in range(2):
                    lo = hp * 128 + hh * 64
                    nmm = TC + (dual_extra is not None)
                    for c in range(TC):
                        nc.tensor.matmul(pa[hh * 64:hh * 64 + 64, c * 128:],
                                         vt[:, c, lo:lo + 64],
                                         ntiles[(hp, c)][:, hh, :],
                                         start=(c == 0), stop=(c + 1 == nmm))
                    if dual_extra is not None:
                        nc.tensor.matmul(pa[hh * 64:hh * 64 + 64, :],
                                         dual_extra["vsb"][si][0:1, lo:lo + 64],
                                         det_bf[:], start=False, stop=True)
                return pa

            def norm(hp, pa):
                pb = psp.tile([128, 512], F32, tag="C", bufs=2, name="ps_rbc")
                nc.tensor.matmul(pb[:], t_sel8[hp][:], rcpb[:], start=True, stop=True)
                rb = work.tile([128, 512], BF16, tag="at_rb", name="at_rb")
                nc.scalar.copy(rb[:], pb[:])
                nc.vector.tensor_tensor(out=odst[:, hp, :], in0=pa[:], in1=rb[:], op=OP.mult)

            pa_prev = attnv(0)
            for hp in range(1, 4):
                pa_cur = attnv(hp)
                norm(hp - 1, pa_prev)
                pa_prev = pa_cur
            norm(3, pa_prev)

    # ================= phase E: embeddings + _ffn per sample =================
    with tc.tile_pool(name="emb", bufs=1) as emb:
        t_ffw1 = wpool.tile([128, C, 2 * D], BF16, tag="wt_w1", bufs=1, name="t_ffw1")
        nc.sync.dma_start(out=t_ffw1[:], in_=w_feat(P["ffw1"]))
        t_ffw2 = wpool.tile([128, 2 * C, D], BF16, tag="wt_w2", bufs=1, name="t_ffw2")
        nc.sync.dma_start(out=t_ffw2[:], in_=w_feat(P["ffw2"]))
        for b in range(BPC):
            gCE = work.tile([128, TC, D], BF16, tag="vtok", name="gCE")
            gCV = work.tile([128, TC, D], BF16, tag="zsq", name="gCV")
            gQD = work.tile([128, TC, D], BF16, tag="hT", name="gQD")
            for tcx in range(TC):
                col = b * TC + tcx
                for tb, dst in (("tb_c", gCE), ("tb_cv", gCV)):
                    nc.gpsimd.indirect_dma_start(
                        out=dst[:, tcx, :], out_offset=None, in_=P[tb][:],
                        in_offset=bass.IndirectOffsetOnAxis(ap=t_icc[:, col:col + 1], axis=0))
                nc.gpsimd.indirect_dma_start(
                    out=gQD[:, tcx, :], out_offset=None, in_=P["tb_qd"][:],
                    in_offset=bass.IndirectOffsetOnAxis(ap=t_icq[:, col:col + 1], axis=0))
            CE = work.tile([128, C, S], BF16, tag="qT", name="CE")
            CV = work.tile([128, C, S], BF16, tag="kT", name="CV")
            QD = work.tile([128, C, S], BF16, tag="attnF", name="QD")
            for si, (src, dst) in enumerate(((gCE, CE), (gCV, CV), (gQD, QD))):
                for dc in range(C):
                    pst = psp.tile([128, 512], BF16, tag="B", bufs=2, name="ps_tr")
                    for tcx in range(TC):
                        nc.tensor.transpose(pst[:, tcx * 128:(tcx + 1) * 128],
                                            src[:, tcx, dc * 128:(dc + 1) * 128],
                                            t_ident[:])
                    if (si + dc) % 2 == 0:
                        nc.vector.tensor_copy(out=dst[:, dc, :], in_=pst[:])
                    else:
                        nc.scalar.copy(dst[:, dc, :], pst[:])
            r_bc = bcast_row(rrow(b), f"r_bc_{b}", work, "r_bc")
            omr_bc = bcast_row(omrow(b), f"omr_bc_{b}", work, "omr_bc")

            posF = work.tile([128, C, S], BF16, tag="z", name="posF")
            negF = work.tile([128, C, S], BF16, tag="xm", name="negF")
            qembF = work.tile([128, C, S], BF16, tag="z2", name="qembF")
            for c in range(C):
                ps_cp1 = psp.tile([128, 512], F32, tag="B", bufs=2, name="ps_cp1")
                nc.tensor.matmul(ps_cp1[:], t_cprow[0:1, c * 128:(c + 1) * 128], omrow(b),
                                 start=True, stop=True)
                ps_cp2 = psp.tile([128, 512], F32, tag="B", bufs=2, name="ps_cp2")
                nc.tensor.matmul(ps_cp2[:], t_cprow[0:1, c * 128:(c + 1) * 128], rrow(b),
                                 start=True, stop=True)
                E1 = work.tile([128, 512], BF16, tag="ln_sq", name="E1")
                nc.vector.scalar_tensor_tensor(out=E1[:], in0=QD[:, c, :], scalar=cp("ivar1", c),
                                               in1=CE[:, c, :], op0=OP.mult, op1=OP.add)
                E0 = work.tile([128, 512], BF16, tag="ln_v", name="E0")
                nc.vector.scalar_tensor_tensor(out=E0[:], in0=QD[:, c, :], scalar=cp("ivar0", c),
                                               in1=CE[:, c, :], op0=OP.mult, op1=OP.add)
                B1 = work.tile([128, 512], BF16, tag="ln_bc", name="B1")
                nc.vector.scalar_tensor_tensor(out=B1[:], in0=E1[:], scalar=cp("inter1", c),
                                               in1=r_bc[:], op0=OP.add, op1=OP.mult)
                B0 = work.tile([128, 512], BF16, tag="ln_t", name="B0")
                nc.vector.scalar_tensor_tensor(out=B0[:], in0=E0[:], scalar=cp("inter0", c),
                                               in1=omr_bc[:], op0=OP.add, op1=OP.mult)
                nc.vector.tensor_tensor(out=posF[:, c, :], in0=B1[:], in1=ps_cp1[:], op=OP.add)
                nc.vector.tensor_tensor(out=negF[:, c, :], in0=B0[:], in1=ps_cp2[:], op=OP.add)
                yv = work.tile([128, 512], BF16, tag="d_t1", name="yv")
                nc.gpsimd.tensor_tensor(out=yv[:], in0=B1[:], in1=B0[:], op=OP.add)
                nc.gpsimd.tensor_tensor(out=Y[b][:, c, :], in0=yv[:], in1=t_peT[:, c, :], op=OP.add)
                qe = work.tile([128, 512], BF16, tag="d_t2", name="qe")
                nc.vector.scalar_tensor_tensor(out=qe[:], in0=QD[:, c, :], scalar=1.0,
                                               in1=CV[:, c, :], op0=OP.mult, op1=OP.mult)
                nc.gpsimd.tensor_tensor(out=qembF[:, c, :], in0=qe[:], in1=CE[:, c, :], op=OP.add)
                nc.gpsimd.tensor_tensor(out=X[b][:, c, :], in0=qembF[:, c, :],
                                        in1=t_peT[:, c, :], op=OP.add)
            nc.sync.dma_start(out=spill[("qemb", b)][:], in_=qembF[:])
            nc.sync.dma_start(out=spill[("qdiff", b)][:], in_=QD[:])

            for src, spname in ((posF, "y1t"), (negF, "y2t")):
                srcT = work.tile([128, TC, D], BF16, tag="vtok", name="srcT")
                for tcx in range(TC):
                    pst2 = psp.tile([128, 512], BF16, tag="B", bufs=2, name="ps_tr2")
                    for dc in range(C):
                        nc.tensor.transpose(pst2[:, dc * 128:(dc + 1) * 128],
                                            src[:, dc, tcx * 128:(tcx + 1) * 128],
                                            t_ident[:])
                    if tcx % 2 == 0:
                        nc.vector.tensor_copy(out=srcT[:, tcx, :], in_=pst2[:])
                    else:
                        nc.scalar.copy(srcT[:, tcx, :], pst2[:])
                hF = work.tile([128, 2 * C, S], BF16, tag="hT", name="hF")
                for m in range(2 * C):
                    psh = psp.tile([128, 512], F32, tag="B", bufs=2, name="psh")
                    for k in range(C):
                        nc.tensor.matmul(psh[:], t_ffw1[:, k, m * 128:(m + 1) * 128],
                                         src[:, k, :], start=(k == 0), stop=(k == C - 1))
                    nc.scalar.activation(hF[:, m, :], psh[:], AF.Gelu, bias=cp("ffb1", m))
                zt = work.tile([128, TC, D], BF16, tag="zsq", name="zt")
                for m in range(TC):
                    psz = psp.tile([128, 512], F32, tag="B", bufs=2, name="psz")
                    for k in range(2 * C):
                        nc.tensor.matmul(psz[:], hF[:, k, m * 128:(m + 1) * 128],
                                         t_ffw2[:, k, :], start=(k == 0), stop=False)
                    nc.tensor.matmul(psz[:], onesrow, rowb(4), start=False, stop=True)
                    zres = work.tile([128, 512], F32, tag="at_denf", name="zres")
                    nc.vector.tensor_tensor(out=zres[:], in0=psz[:], in1=srcT[:, m, :],
                                            op=OP.add)
                    st6 = work.tile([128, 6], F32, tag="st6", name="st6")
                    nc.vector.bn_stats(st6[:], zres[:])
                    mv = work.tile([128, 2], F32, tag="mv", name="mv")
                    nc.vector.bn_aggr(mv[:], st6[:])
                    sv = work.tile([128, 1], F32, tag="sv", name="sv")
                    nc.scalar.activation(sv[:], mv[:, 1:2], AF.Sqrt, bias=cp("eps12"))
                    rs = work.tile([128, 1], F32, tag="rs", name="rs")
                    nc.vector.reciprocal_approx_fast(out=rs[:], in_=sv[:])
                    tnorm = work.tile([128, 512], BF16, tag="ln_rstdb", name="tnorm")
                    nc.vector.tensor_scalar(out=tnorm[:], in0=zres[:], scalar1=mv[:, 0:1],
                                            scalar2=rs[:, 0:1], op0=OP.subtract, op1=OP.mult)
                    tg = work.tile([128, 512], BF16, tag="at_rb", name="tg")
                    nc.vector.tensor_tensor(out=tg[:], in0=tnorm[:], in1=t_ffg_bc[:], op=OP.mult)
                    nc.gpsimd.tensor_tensor(out=zt[:, m, :], in0=tg[:], in1=t_ffb_bc[:], op=OP.add)
                nc.sync.dma_start(out=spill[(spname, b)][:], in_=zt[:])

    # ================= phase B: transformer blocks =================
    for l in range(NB):
        WT = {}
        for nm, nk, nn, nb in (("wq", C, D, 2), ("wk", C, D, 2), ("wv", C, D, 2),
                               ("wo", C, D, 2), ("w1", C, 2 * D, 1), ("w2", 2 * C, D, 1)):
            WT[nm] = wpool.tile([128, nk, nn], BF16, tag=f"wt_{nm}", bufs=nb,
                                name=f"wt_{nm}_{l}")
            nc.sync.dma_start(out=WT[nm][:], in_=w_feat(P[f"{nm}{l}"]))
        for b in range(BPC):
            qT = work.tile([128, C, S], BF16, tag="qT", name="qT")
            kT = work.tile([128, C, S], BF16, tag="kT", name="kT")
            linear_feat(qT, X[b], WT["wq"], C, C, bias_name=f"bq{l}")
            linear_feat(kT, X[b], WT["wk"], C, C, bias_name=f"bk{l}")
            vtok = work.tile([128, TC, D], BF16, tag="vtok", name="vtok")
            for m in range(TC):
                psv = psp.tile([128, 512], F32, tag="B", bufs=2, name="psv")
                for k in range(C):
                    nc.tensor.matmul(psv[:], Y[b][:, k, m * 128:(m + 1) * 128],
                                     WT["wv"][:, k, :], start=(k == 0), stop=False)
                nc.tensor.matmul(psv[:], onesrow, rowb(l), start=False, stop=True)
                if m % 2 == 1:
                    nc.scalar.copy(vtok[:, m, :], psv[:])
                else:
                    nc.vector.tensor_copy(out=vtok[:, m, :], in_=psv[:])
            attnF = work.tile([128, C, S], BF16, tag="attnF", name="attnF")
            attention(qT, kT, vtok, attnF, b)
            z = work.tile([128, C, S], BF16, tag="z", name="z")
            zsq1 = work.tile([128, C, S], BF16, tag="zsq", name="zsq1")
            linear_feat(z, attnF, WT["wo"], C, C, bias_name=f"bo{l}", resid=X[b],
                        square_dst=zsq1)
            xm = work.tile([128, C, S], BF16, tag="xm", name="xm")
            ln_feature(xm, z, zsq1, f"ln1g{l}", f"ln1gn{l}", f"ln1b{l}", "eps5")
            hT = work.tile([128, 2 * C, S], BF16, tag="hT", name="hT")
            linear_feat(hT, xm, WT["w1"], C, 2 * C, bias_name=f"b1_{l}", relu=True)
            z2 = work.tile([128, C, S], BF16, tag="z2", name="z2")
            zsq2 = work.tile([128, C, S], BF16, tag="zsq", name="zsq2")
            linear_feat(z2, hT, WT["w2"], 2 * C, C, bias_name=f"b2_{l}", resid=xm,
                        square_dst=zsq2)
            ln_feature(X[b], z2, zsq2, f"ln2g{l}", f"ln2gn{l}", f"ln2b{l}", "eps5")

    # ================= phase D: dual attention + head =================
    t_pw1g = []
    for gi, tg in enumerate(("wt_wq", "wt_wk", "wt_wv")):
        tt = wpool.tile([128, C, D], BF16, tag=tg, bufs=2, name=f"t_pw1_{gi}")
        nc.sync.dma_start(out=tt[:], in_=w_feat(P["pw1"])[:, gi * C:(gi + 1) * C, :])
        t_pw1g.append(tt)
    t_pw2 = wpool.tile([128, C, 1], BF16, name="t_pw2")
    nc.sync.dma_start(out=t_pw2[:], in_=P["pw2"][:].rearrange("(k p) m -> p k m", p=128))
    for b in range(BPC):
        y1t = work.tile([128, TC, D], BF16, tag="vtok", name="y1t")
        nc.sync.dma_start(out=y1t[:], in_=spill[("y1t", b)][:])
        y2t = work.tile([128, TC, D], BF16, tag="zsq", name="y2t")
        nc.sync.dma_start(out=y2t[:], in_=spill[("y2t", b)][:])
        vsb = []
        for si, vt in enumerate((y1t, y2t)):
            psvs = psp.tile([1, 512], F32, tag="C", bufs=2, name=f"ps_vs{si}")
            for c in range(TC):
                nc.tensor.matmul(psvs[:], onescol, vt[:, c, :],
                                 start=(c == 0), stop=(c == TC - 1))
            v1 = work.tile([1, 512], BF16, tag=f"vsb{si}", name=f"vsb{si}")
            nc.vector.tensor_copy(out=v1[:], in_=psvs[:])
            vsb.append(v1)
        o1 = work.tile([128, C, S], BF16, tag="attnF", name="o1")
        o2 = work.tile([128, C, S], BF16, tag="z2", name="o2")
        attention(X[b], X[b], y1t, o1, b,
                  dual_extra=dict(vtok2=y2t, outdst2=o2, vsb=vsb))
        qdf = work.tile([128, C, S], BF16, tag="qT", name="qdf")
        nc.sync.dma_start(out=qdf[:], in_=spill[("qdiff", b)][:])
        qem = work.tile([128, C, S], BF16, tag="kT", name="qem")
        nc.sync.dma_start(out=qem[:], in_=spill[("qemb", b)][:])
        xf = work.tile([128, C, S], BF16, tag="xm", name="xf")
        od = work.tile([128, C, S], BF16, tag="z", name="od")
        for c in range(C):
            t1 = work.tile([128, 512], BF16, tag="d_t1", name="d_t1")
            nc.vector.tensor_tensor(out=t1[:], in0=o1[:, c, :], in1=o2[:, c, :], op=OP.add)
            t2 = work.tile([128, 512], BF16, tag="d_t2", name="d_t2")
            nc.vector.tensor_tensor(out=t2[:], in0=X[b][:, c, :], in1=t1[:], op=OP.subtract)
            nc.gpsimd.tensor_tensor(out=xf[:, c, :], in0=t2[:], in1=qdf[:, c, :], op=OP.subtract)
            nc.gpsimd.tensor_tensor(out=od[:, c, :], in0=o1[:, c, :], in1=o2[:, c, :], op=OP.subtract)
        hp_ = work.tile([128, C, S], BF16, tag="hT", name="hp_")
        for m in range(C):
            psh = psp.tile([128, 512], F32, tag="B", bufs=2, name="psph")
            i = 0
            for g_i, gsrc in enumerate((xf, qem, od)):
                for k in range(C):
                    i += 1
                    nc.tensor.matmul(psh[:], t_pw1g[g_i][:, k, m * 128:(m + 1) * 128],
                                     gsrc[:, k, :], start=(i == 1), stop=(i == 3 * C))
            nc.vector.tensor_scalar(out=hp_[:, m, :], in0=psh[:], scalar1=cp("pb1", m),
                                    scalar2=0.0, op0=OP.add, op1=OP.max)
        pso = psp.tile([1, 512], F32, tag="C", bufs=2, name="ps_out")
        for k in range(C):
            nc.tensor.matmul(pso[:], t_pw2[:, k, 0:1], hp_[:, k, :],
                             start=(k == 0), stop=(k == C - 1))
        ofin = work.tile([1, 512], F32, tag="ofin", name="ofin")
        nc.scalar.activation(ofin[:], pso[:], AF.Sigmoid, bias=t_pb2[0:1, 0:1])
        nc.sync.dma_start(out=out[b:b + 1, :], in_=ofin[:])

    ctx.close()


# ------------------------------------------------------------------ driver --
_CACHE = {}


def _get_runner(nc):
    """Build a reusable sharded jit callable for the finalized Bass module."""
    if "runner" in _CACHE:
        return _CACHE["runner"]
    import jax
    import concourse.mybir as mybir_
    from concourse.bass2jax import install_neuronx_cc_hook, _bass_exec_p, partition_id_tensor
    from jax.sharding import Mesh, PartitionSpec
    from jax.experimental.shard_map import shard_map

    install_neuronx_cc_hook()
    partition_name = nc.partition_id_tensor.name if nc.partition_id_tensor else None
    in_names, out_names, out_avals, zero_outs = [], [], [], []
    for alloc in nc.m.functions[0].allocations:
        if not isinstance(alloc, mybir_.MemoryLocationSet):
            continue
        name = alloc.memorylocations[0].name
        if alloc.kind == "ExternalInput":
            if name != partition_name:
                in_names.append(name)
        elif alloc.kind == "ExternalOutput":
            shape = tuple(alloc.tensor_shape)
            dtype = mybir_.dt.np(alloc.dtype)
            out_names.append(name)
            out_avals.append(jax.core.ShapedArray(shape, dtype))
            zero_outs.append(np.zeros(shape, dtype))
    n_params = len(in_names)
    n_outs = len(out_avals)
    all_in_names = list(in_names) + list(out_names)
    if partition_name is not None:
        all_in_names.append(partition_name)
    donate = tuple(range(n_params, n_params + n_outs))

    def _body(*args):
        operands = list(args)
        if partition_name is not None:
            operands.append(partition_id_tensor())
        outs = _bass_exec_p.bind(
            *operands, out_avals=tuple(out_avals), in_names=tuple(all_in_names),
            out_names=tuple(out_names), lowering_input_output_aliases=(),
            sim_require_finite=True, sim_require_nnan=True, nc=nc)
        return tuple(outs)

    devices = jax.devices()[:NC_CORES]
    mesh = Mesh(np.asarray(devices), ("core",))
    in_specs = (PartitionSpec("core"),) * (n_params + n_outs)
    out_specs = (PartitionSpec("core"),) * n_outs
    sharded = jax.jit(
        shard_map(_body, mesh=mesh, in_specs=in_specs, out_specs=out_specs,
                  check_rep=False),
        donate_argnums=donate, keep_unused=True)

    runner = dict(sharded=sharded, in_names=in_names, out_names=out_names,
                  out_avals=out_avals, zero_outs=zero_outs, mesh=mesh)
    _CACHE["runner"] = runner
    return runner


def _run(nc, in_maps):
    r = _get_runner(nc)
    concat_in = [
        np.concatenate([np.asarray(in_maps[c][name]) for c in range(NC_CORES)], axis=0)
        for name in r["in_names"]
    ]
    concat_zeros = [
        np.zeros((NC_CORES * z.shape[0], *z.shape[1:]), z.dtype)
        for z in r["zero_outs"]
    ]
    out_arrs = r["sharded"](*concat_in, *concat_zeros)
    outs = {}
    for i, name in enumerate(r["out_names"]):
        outs[name] = np.asarray(out_arrs[i]).reshape(
            NC_CORES, *r["out_avals"][i].shape)
    return outs


def _prep_all(params, question_seq, concept_seq, correctness_seq, counter_mask_seq):
    def tonp(o):
        if isinstance(o, dict):
            return {k: tonp(v) for k, v in o.items()}
        if isinstance(o, list):
            return [tonp(v) for v in o]
        return np.asarray(o)

    params = tonp(params)
    g = prep_host(params)
    cpidx = g.pop("_cpidx")
    if "nc" not in _CACHE:
        nc = build(cpidx, g["cpack"].shape[1])
        nc.finalize()
        _CACHE["nc"] = nc
    in_maps = []
    for core in range(NC_CORES):
        m = dict(g)
        m.update(prep_core(question_seq, concept_seq, correctness_seq,
                           counter_mask_seq, core))
        in_maps.append(m)
    return _CACHE["nc"], in_maps


def kernel(params, question_seq, concept_seq, correctness_seq, counter_mask_seq):
    nc, in_maps = _prep_all(params, question_seq, concept_seq,
                            correctness_seq, counter_mask_seq)
    outs = _run(nc, in_maps)
    return np.concatenate([outs["out"][c] for c in range(NC_CORES)],
                          axis=0).astype(np.float32)
